# revision 20
# baseline (speedup 1.0000x reference)
"""CapsMaxPool Trainium2 kernel.

x: [B=64, H=64, W=64, C=32, A=8] fp32 capsules. For each 2x2 spatial window
and capsule c, pick the candidate position whose capsule vector has the
largest L2 norm (first-max-wins over the window in row-major (ph, pw) order)
and emit that 8-atom vector. Output: [B, 32, 32, 32, 8].

Strategy (per core; batch sharded 8 ways -> 8 examples/core):
  - Layout: spatial rows on SBUF partitions, (capsule, atom) on the free dim.
    One partition row = one (b, ho) output row.
  - The four window candidates are DMA'd as slices of one SBUF tile; the
    paired load rides both pw candidates in one transfer (2 KiB contiguous
    DRAM chunks).
  - ScalarE squares everything (fp32; fp16/bf16 squares were measured to
    flip 346/2691 argmax decisions vs the fp32 reference, while the fp32
    pipeline is bit-exact and stable to +-6 ulp of any summation order).
  - Atom sums via a pairwise add tree on VectorE (tensor-tensor adds read
    2 elems/cycle via two SBUF ports vs tensor_reduce's 1).
  - VectorE builds strictly-greater masks against the running max
    (first-max-wins, identical to jnp.argmax), then copy_predicated
    overwrites the candidate-0 slice in place, broadcasting each (wo, c)
    mask over the 8 atoms via a step-0 AP dim.
"""

import numpy as np

import concourse.bass as bass
import concourse.tile as tile
from concourse import mybir
from concourse.bass_utils import run_bass_kernel_spmd

B, H, W, C, A = 64, 64, 64, 32, 8
PH = PW = 2
NCORES = 8
BL = B // NCORES          # batches per core
Ho, Wo = H // PH, W // PW
CA = C * A                # 256
R = BL * Ho               # 256 partition rows per core ((b, ho) flattened)
NRT = R // 128            # row tiles

F32 = mybir.dt.float32


def _split_multi_waits(nc):
    """Walrus on this toolchain encodes at most ONE sync wait per
    instruction; Tile attaches several. Hoist all-but-one wait into
    standalone InstEventSemaphore ops just before the instruction (same
    engine stream position => identical semantics)."""
    for fn in nc.m.functions:
        for bb in fn.blocks:
            new = []
            for ins in bb.instructions:
                si = ins.sync_info
                if si is not None and si.on_wait and len(si.on_wait) > 1:
                    waits = list(si.on_wait)
                    for j, w in enumerate(waits[:-1]):
                        new.append(mybir.InstEventSemaphore(
                            name=f"{ins.name}-hw{j}",
                            engine=ins.engine,
                            ins=[], outs=[],
                            sync_info=mybir.SyncInfo(on_wait=[w], on_update=[]),
                        ))
                    ins.sync_info = mybir.SyncInfo(
                        on_wait=[waits[-1]], on_update=list(si.on_update)
                    )
                new.append(ins)
            bb.instructions = new


def _bcast_atoms(ap):
    """View an AP with an extra step-0 trailing dim of size A."""
    return bass.AP(tensor=ap.tensor, offset=ap.offset, ap=list(ap.ap) + [[0, A]])


def _group(nc, big, small, xv, ov, r0, w0, cfg, dst_out=None):
    NP = PH * PW
    WCH = cfg["wch"]
    if cfg.get("contig"):
        # One DMA per group: each partition row loads its full window span
        # (both ph rows x 2*WCH w positions) as two contiguous 8 KiB DRAM
        # chunks. Candidates are strided views of the loaded tile.
        xvc = cfg["xvc"]
        xq = big.tile(
            [128, PH, 2 * WCH, CA], F32, name="xq",
            bufs=cfg.get("load_bufs"),
        )
        if cfg.get("split4"):
            # 4 DMAs: one per (ph, w-half) — 4 KiB contiguous chunks with
            # candidate-level dependency granularity. Optionally spread
            # across both HWDGE rings (SP + Activation).
            for ph in range(PH):
                for wh in range(2):
                    eng = (
                        nc.scalar
                        if cfg.get("in_split_rings") and wh == 1
                        else nc.sync
                    )
                    eng.dma_start(
                        out=xq[:, ph, wh * WCH : (wh + 1) * WCH, :],
                        in_=xvc[
                            r0 : r0 + 128, ph,
                            2 * w0 + wh * WCH : 2 * w0 + (wh + 1) * WCH, :,
                        ],
                    )
        elif cfg.get("split_ph"):
            for ph in range(PH):
                nc.sync.dma_start(
                    out=xq[:, ph],
                    in_=xvc[r0 : r0 + 128, ph, 2 * w0 : 2 * (w0 + WCH), :],
                )
        else:
            nc.sync.dma_start(
                out=xq,
                in_=xvc[r0 : r0 + 128, :, 2 * w0 : 2 * (w0 + WCH), :],
            )
        xqv = xq[:].rearrange("p ph (wo pw) ca -> p ph pw wo ca", pw=PW)
        cand = lambda p: xqv[:, p // PW, p % PW]
    elif cfg.get("pair_load"):
        # xc laid out [part, w, cand, ca] so one DMA per ph row carries both
        # pw candidates: the DRAM side is then fully contiguous 8 KiB per
        # partition row, and the SBUF side stays a 3-dim AP.
        xcw = big.tile([128, WCH, NP, CA], F32, name="xcw")
        for ph in range(PH):
            nc.sync.dma_start(
                out=xcw[:, :, 2 * ph : 2 * ph + 2, :],
                in_=xv[r0 : r0 + 128, ph, :, w0 : w0 + WCH, :].rearrange(
                    "p pw w ca -> p w pw ca"
                ),
            )
        cand = lambda p: xcw[:, :, p, :]
    else:
        xc = big.tile(
            [128, NP, WCH, CA], F32, name="xc", bufs=cfg.get("load_bufs")
        )
        for p in range(NP):
            ph, pw = divmod(p, PW)
            nc.sync.dma_start(
                out=xc[:, p],
                in_=xv[r0 : r0 + 128, ph, pw, w0 : w0 + WCH, :],
            )
        cand = lambda p: xc[:, p]

    if cfg.get("loads_only"):
        return

    if cfg.get("dma_only"):
        oe = (nc.gpsimd if cfg.get("out_gps_ring")
              else nc.scalar if cfg.get("out_act_ring") else nc.sync)
        if cfg.get("inplace"):
            oe.dma_start(
                out=ov[r0 : r0 + 128, w0 : w0 + WCH, :], in_=cand(0)
            )
        else:
            out_t = big.tile([128, WCH, CA], F32, name="out_t")
            nc.scalar.copy(out_t, cand(0))
            oe.dma_start(out=ov[r0 : r0 + 128, w0 : w0 + WCH, :], in_=out_t)
        return

    ENG = {"act": nc.scalar, "dve": nc.vector, "gps": nc.gpsimd}

    probe = cfg.get("probe", "")

    sq = big.tile(
        [128, NP, WCH, CA], F32, name="sq", bufs=cfg.get("sq_bufs")
    )
    sq_engs = cfg.get("sq_engs", "act,act,act,act").split(",")
    sq = big.tile(
        [128, NP, WCH, CA], F32, name="sq", bufs=cfg.get("sq_bufs")
    )
    if probe != "no_sq":
        if cfg.get("sq_whole") and cfg.get("contig"):
            # One wide ACT square per ph half (2048 elems vs 4x1024):
            # strided read de-interleaves (wo, pw) so sq stays
            # candidate-major; issued as soon as that ph's loads land.
            for ph in range(PH):
                nc.scalar.activation(
                    sq[:, 2 * ph : 2 * ph + 2],
                    xq[:, ph].rearrange(
                        "p (wo pw) ca -> p pw wo ca", pw=PW, wo=WCH
                    ),
                    mybir.ActivationFunctionType.Square,
                )
        else:
            for p in range(NP):
                if sq_engs[p] == "act":
                    nc.scalar.activation(
                        sq[:, p], cand(p), mybir.ActivationFunctionType.Square
                    )
                else:
                    ENG[sq_engs[p]].tensor_tensor(
                        sq[:, p], cand(p), cand(p), mybir.AluOpType.mult
                    )
    sq_cand = sq[:]

    s = small.tile([128, NP, WCH, C], F32, name="s")
    norm = cfg.get("norm", "tree_dve")
    if probe == "no_tree":
        # perf probe: fake norms = one DVE copy of C elems per cand
        sv = sq_cand.rearrange("p q w (c a) -> p q w c a", a=A)
        nc.vector.tensor_scalar_add(s, sv[:, :, :, :, 0], 0.0)
    elif norm in ("tree_dve", "tree_gps"):
        te = cfg.get("tree_engs")
        te = (te.split(",") if te
              else ["gps", "dve", "dve"] if norm == "tree_gps"
              else ["dve", "dve", "dve"])
        if probe == "no_sq":
            # perf probe: tree reads the raw load tile (garbage norms)
            sqt = xq[:].rearrange(
                "p ph (q2 w) ca -> p (ph q2) w ca", q2=2
            )
        else:
            sqt = sq_cand
        t4 = small.tile([128, NP, WCH, C, A // 2], F32, name="t4")
        t2 = small.tile([128, NP, WCH, C, A // 4], F32, name="t2")
        if cfg.get("tree_halves"):
            # halves layout: contiguous runs (GPS-friendly), any summation
            # order is valid for the norm
            sqv = sqt.rearrange("p q w (c hl a4) -> p q w c hl a4", hl=2, a4=A // 2)
            kg = cfg.get("l1_gps_cands")
            if kg is None:
                ENG[te[0]].tensor_add(t4, sqv[:, :, :, :, 0], sqv[:, :, :, :, 1])
            else:
                # first NP-kg candidates on DVE, last kg on GPS
                kd = NP - kg
                if kd:
                    nc.vector.tensor_add(
                        t4[:, :kd], sqv[:, :kd, :, :, 0], sqv[:, :kd, :, :, 1]
                    )
                if kg:
                    nc.gpsimd.tensor_add(
                        t4[:, kd:], sqv[:, kd:, :, :, 0], sqv[:, kd:, :, :, 1]
                    )
            t4v = t4[:].rearrange("p q w c (hl a2) -> p q w c hl a2", hl=2, a2=A // 4)
            ENG[te[1]].tensor_add(t2, t4v[:, :, :, :, 0], t4v[:, :, :, :, 1])
            t2v = t2[:].rearrange("p q w c (hl a1) -> p q w c hl a1", hl=2, a1=1)
            ENG[te[2]].tensor_add(
                s, t2v[:, :, :, :, 0, 0], t2v[:, :, :, :, 1, 0]
            )
        else:
            sqv = sqt.rearrange(
                "p q w (c a2 two) -> p q w c a2 two", a2=A // 2, two=2
            )
            ENG[te[0]].tensor_add(
                t4, sqv[:, :, :, :, :, 0], sqv[:, :, :, :, :, 1]
            )
            t4v = t4[:].rearrange("p q w c (b2 two) -> p q w c b2 two", two=2)
            ENG[te[1]].tensor_add(
                t2, t4v[:, :, :, :, :, 0], t4v[:, :, :, :, :, 1]
            )
            t2v = t2[:].rearrange("p q w c (b1 two) -> p q w c b1 two", two=2)
            ENG[te[2]].tensor_add(s, t2v[:, :, :, :, 0, 0], t2v[:, :, :, :, 0, 1])
    else:  # plain reduce on DVE
        nc.vector.tensor_reduce(
            s,
            sq[:].rearrange("p q w (c a) -> p q w c a", a=A),
            axis=mybir.AxisListType.X,
            op=mybir.AluOpType.add,
        )

    if cfg.get("sel_merge"):
        # Pairwise select tree: m01/m23 pick within pairs, mf picks the
        # pair (strict > everywhere keeps first-max-wins tie-breaking).
        me = ENG[cfg.get("mask_eng", "dve")]
        mdt = mybir.dt.uint8
        m01 = small.tile([128, WCH, C], mdt, name="m01")
        m23 = small.tile([128, WCH, C], mdt, name="m23")
        mf = small.tile([128, WCH, C], mdt, name="mf")
        r01 = small.tile([128, WCH, C], F32, name="r01")
        r23 = small.tile([128, WCH, C], F32, name="r23")
        me.tensor_tensor(m01, s[:, 1], s[:, 0], mybir.AluOpType.is_gt)
        me.tensor_tensor(m23, s[:, 3], s[:, 2], mybir.AluOpType.is_gt)
        me.tensor_max(r01, s[:, 0], s[:, 1])
        me.tensor_max(r23, s[:, 2], s[:, 3])
        me.tensor_tensor(mf, r23, r01, mybir.AluOpType.is_gt)
        v = lambda ap: ap.rearrange("p w (c a) -> p w c a", a=A)
        sel01 = big.tile([128, WCH, CA], F32, name="sel01",
                         bufs=cfg.get("sel_bufs", 2))
        sel23 = big.tile([128, WCH, CA], F32, name="sel23",
                         bufs=cfg.get("sel_bufs", 2))
        nc.vector.select(
            v(sel01[:]), _bcast_atoms(m01[:]), v(cand(1)), v(cand(0))
        )
        nc.vector.select(
            v(sel23[:]), _bcast_atoms(m23[:]), v(cand(3)), v(cand(2))
        )
        if dst_out is not None:
            dst_slice = dst_out
        else:
            out_t = big.tile([128, WCH, CA], F32, name="out_t")
            dst_slice = out_t[:]
        nc.vector.select(
            v(dst_slice), _bcast_atoms(mf[:]), v(sel23[:]), v(sel01[:])
        )
        if dst_out is None:
            out_eng = (nc.gpsimd if cfg.get("out_gps_ring")
                       else nc.scalar if cfg.get("out_act_ring") else nc.sync)
            out_eng.dma_start(
                out=ov[r0 : r0 + 128, w0 : w0 + WCH, :], in_=dst_slice
            )
        return

    # Strict-greater masks vs the running max -> first-max-wins.
    me = ENG[cfg.get("mask_eng", "dve")]
    m = [
        small.tile([128, WCH, C], mybir.dt.uint8, name=f"mask{i}", tag=f"mask{i}")
        for i in range(3)
    ]
    r01 = small.tile([128, WCH, C], F32, name="r01")
    r012 = small.tile([128, WCH, C], F32, name="r012")
    me.tensor_tensor(m[0], s[:, 1], s[:, 0], mybir.AluOpType.is_gt)
    if probe == "no_masks":
        m[1] = m[0]
        m[2] = m[0]
    else:
        me.tensor_max(r01, s[:, 0], s[:, 1])
        me.tensor_tensor(m[1], s[:, 2], r01, mybir.AluOpType.is_gt)
        me.tensor_max(r012, r01, s[:, 2])
        me.tensor_tensor(m[2], s[:, 3], r012, mybir.AluOpType.is_gt)

    def _copy(dst, src):
        ce = cfg.get("copy_eng", "act")
        if ce == "act":
            nc.scalar.copy(dst, src)
        else:
            ENG[ce].tensor_scalar_add(dst, src, 0.0)

    # Select: overwrite the baseline wherever a later candidate strictly
    # beats the running max (mask broadcast over the 8 atoms via step-0).
    if dst_out is not None:
        dst_slice = dst_out
        _copy(dst_slice, cand(0))
    elif cfg.get("inplace"):
        dst_slice = cand(0)
    else:
        out_t = big.tile([128, WCH, CA], F32, name="out_t")
        _copy(out_t, cand(0))
        dst_slice = out_t[:]
    dst = dst_slice.rearrange("p w (c a) -> p w c a", a=A)
    if probe != "no_preds":
        for p in range(1, NP):
            nc.vector.copy_predicated(
                dst,
                _bcast_atoms(m[p - 1][:]),
                cand(p).rearrange("p w (c a) -> p w c a", a=A),
            )

    if dst_out is None:
        out_eng = (nc.gpsimd if cfg.get("out_gps_ring")
                   else nc.scalar if cfg.get("out_act_ring") else nc.sync)
        out_eng.dma_start(
            out=ov[r0 : r0 + 128, w0 : w0 + WCH, :], in_=dst_slice
        )



def _slab_sel(nc, big, small, xv, ov, r0, wq, cfg):
    """Slab-level sel_merge: per group, compute s (norms) and pairwise
    selects into slab-wide tiles; then batched masks + ONE final select
    over the whole slab + one store."""
    WCH = cfg["wch"]
    NP = PH * PW
    sb = cfg["store_batch"]
    xvc = cfg["xvc"]
    lb = cfg.get("load_bufs")

    s_slab = small.tile([128, NP, sb, WCH, C], F32, name="s_slab")
    sel01 = big.tile([128, sb, WCH, CA], F32, name="sel01",
                     bufs=cfg.get("slab_bufs", 2))
    sel23 = big.tile([128, sb, WCH, CA], F32, name="sel23",
                     bufs=cfg.get("slab_bufs", 2))
    m01 = small.tile([128, sb, WCH, C], mybir.dt.uint8, name="m01")
    m23 = small.tile([128, sb, WCH, C], mybir.dt.uint8, name="m23")
    mf = small.tile([128, sb, WCH, C], mybir.dt.uint8, name="mf")
    r01 = small.tile([128, sb, WCH, C], F32, name="r01")
    r23 = small.tile([128, sb, WCH, C], F32, name="r23")
    oslab = big.tile([128, sb, WCH, CA], F32, name="oslab",
                     bufs=cfg.get("slab_bufs", 2))

    v = lambda ap: ap.rearrange("p w (c a) -> p w c a", a=A)
    vs = lambda ap: ap.rearrange("p b w (c a) -> p b w c a", a=A)
    cands = []
    for sub in range(sb):
        w0 = (wq * sb + sub) * WCH
        xq = big.tile([128, PH, 2 * WCH, CA], F32, name="xq", bufs=lb)
        for ph in range(PH):
            for wh in range(2):
                nc.sync.dma_start(
                    out=xq[:, ph, wh * WCH : (wh + 1) * WCH, :],
                    in_=xvc[
                        r0 : r0 + 128, ph,
                        2 * w0 + wh * WCH : 2 * w0 + (wh + 1) * WCH, :,
                    ],
                )
        xqv = xq[:].rearrange("p ph (wo pw) ca -> p ph pw wo ca", pw=PW)
        cand = lambda p, _x=xqv: _x[:, p // PW, p % PW]
        cands.append(cand)

        sq = big.tile([128, NP, WCH, CA], F32, name="sq",
                      bufs=cfg.get("sq_bufs"))
        for ph in range(PH):
            nc.scalar.activation(
                sq[:, 2 * ph : 2 * ph + 2],
                xq[:, ph].rearrange(
                    "p (wo pw) ca -> p pw wo ca", pw=PW, wo=WCH
                ),
                mybir.ActivationFunctionType.Square,
            )
        sqv = sq[:].rearrange(
            "p q w (c hl a4) -> p q w c hl a4", hl=2, a4=A // 2
        )
        t4 = small.tile([128, NP, WCH, C, A // 2], F32, name="t4",
                        bufs=cfg.get("tree_bufs", 2))
        kg = cfg.get("l1_gps_cands") or 0
        kd = NP - kg
        if kd:
            nc.vector.tensor_add(
                t4[:, :kd], sqv[:, :kd, :, :, 0], sqv[:, :kd, :, :, 1]
            )
        if kg:
            nc.gpsimd.tensor_add(
                t4[:, kd:], sqv[:, kd:, :, :, 0], sqv[:, kd:, :, :, 1]
            )
        t4v = t4[:].rearrange("p q w c (hl a2) -> p q w c hl a2",
                              hl=2, a2=A // 4)
        t2 = small.tile([128, NP, WCH, C, A // 4], F32, name="t2")
        nc.vector.tensor_add(t2, t4v[:, :, :, :, 0], t4v[:, :, :, :, 1])
        t2v = t2[:].rearrange("p q w c (hl a1) -> p q w c hl a1",
                              hl=2, a1=1)
        nc.vector.tensor_add(
            s_slab[:, :, sub], t2v[:, :, :, :, 0, 0], t2v[:, :, :, :, 1, 0]
        )

    sv = s_slab[:]
    me = nc.vector
    me.tensor_tensor(m01, sv[:, 1], sv[:, 0], mybir.AluOpType.is_gt)
    me.tensor_tensor(m23, sv[:, 3], sv[:, 2], mybir.AluOpType.is_gt)
    me.tensor_max(r01, sv[:, 0], sv[:, 1])
    me.tensor_max(r23, sv[:, 2], sv[:, 3])
    me.tensor_tensor(mf, r23, r01, mybir.AluOpType.is_gt)

    def bca(ap):
        return bass.AP(
            tensor=ap.tensor, offset=ap.offset, ap=list(ap.ap) + [[0, A]]
        )

    for sub in range(sb):
        nc.vector.select(
            v(sel01[:, sub]), bca(m01[:, sub]),
            v(cands[sub](1)), v(cands[sub](0)),
        )
        nc.vector.select(
            v(sel23[:, sub]), bca(m23[:, sub]),
            v(cands[sub](3)), v(cands[sub](2)),
        )
    nc.vector.select(vs(oslab[:]), bca(mf[:]), vs(sel23[:]), vs(sel01[:]))
    oe = nc.scalar if cfg.get("out_act_ring") else nc.sync
    oe.dma_start(
        out=ov[r0 : r0 + 128, wq * sb * WCH : (wq + 1) * sb * WCH, :],
        in_=oslab,
    )


def _slab_v4(nc, big, small, xv, ov, r0, wq, cfg):
    """v4: slab-wide squares/tree/masks (one DVE instr per tree level per
    slab), per-group loads kept at 4x512KiB for DMA concurrency, merge via
    ACT copy + 3 preds per group, store on the ACT ring."""
    WCH = cfg["wch"]
    NP = PH * PW
    sb = cfg["store_batch"]
    Q = sb * NP
    xvc = cfg["xvc"]

    xqs = []
    sq_slab = big.tile([128, Q, WCH, CA], F32, name="sq_slab",
                       bufs=cfg.get("sq_bufs", 2))
    oslab = big.tile([128, sb, WCH, CA], F32, name="oslab",
                     bufs=cfg.get("slab_bufs", 2))
    for sub in range(sb):
        w0 = (wq * sb + sub) * WCH
        xq = big.tile([128, PH, 2 * WCH, CA], F32, name="xq",
                      bufs=cfg.get("load_bufs"))
        for ph in range(PH):
            for wh in range(2):
                nc.sync.dma_start(
                    out=xq[:, ph, wh * WCH : (wh + 1) * WCH, :],
                    in_=xvc[
                        r0 : r0 + 128, ph,
                        2 * w0 + wh * WCH : 2 * w0 + (wh + 1) * WCH, :,
                    ],
                )
        xqs.append(xq)
        for ph in range(PH):
            nc.scalar.activation(
                sq_slab[:, sub * NP + 2 * ph : sub * NP + 2 * ph + 2],
                xq[:, ph].rearrange(
                    "p (wo pw) ca -> p pw wo ca", pw=PW, wo=WCH
                ),
                mybir.ActivationFunctionType.Square,
            )

    # slab-wide pairwise tree (halves order; any summation order is valid)
    sqv = sq_slab[:].rearrange(
        "p q w (c hl a4) -> p q w c hl a4", hl=2, a4=A // 2
    )
    t4 = small.tile([128, Q, WCH, C, A // 2], F32, name="t4",
                    bufs=cfg.get("tree_bufs", 2))
    nc.vector.tensor_add(t4, sqv[:, :, :, :, 0], sqv[:, :, :, :, 1])
    t4v = t4[:].rearrange("p q w c (hl a2) -> p q w c hl a2",
                          hl=2, a2=A // 4)
    t2 = small.tile([128, Q, WCH, C, A // 4], F32, name="t2",
                    bufs=cfg.get("tree_bufs", 2))
    nc.vector.tensor_add(t2, t4v[:, :, :, :, 0], t4v[:, :, :, :, 1])
    t2v = t2[:].rearrange("p q w c (hl a1) -> p q w c hl a1", hl=2, a1=1)
    s_slab = small.tile([128, Q, WCH, C], F32, name="s_slab", bufs=2)
    nc.vector.tensor_add(
        s_slab, t2v[:, :, :, :, 0, 0], t2v[:, :, :, :, 1, 0]
    )

    # batched masks over all sb groups: view s as [p, sb, q, w, c]
    sv = s_slab[:].rearrange("p (sb q) w c -> p sb q w c", sb=sb, q=NP)
    mdt = mybir.dt.uint8
    m = [small.tile([128, sb, WCH, C], mdt, name=f"m{i}", bufs=2)
         for i in range(3)]
    r01 = small.tile([128, sb, WCH, C], F32, name="r01", bufs=2)
    r012 = small.tile([128, sb, WCH, C], F32, name="r012", bufs=2)
    nc.vector.tensor_tensor(m[0], sv[:, :, 1], sv[:, :, 0],
                            mybir.AluOpType.is_gt)
    nc.vector.tensor_max(r01, sv[:, :, 0], sv[:, :, 1])
    nc.vector.tensor_tensor(m[1], sv[:, :, 2], r01, mybir.AluOpType.is_gt)
    nc.vector.tensor_max(r012, r01, sv[:, :, 2])
    nc.vector.tensor_tensor(m[2], sv[:, :, 3], r012, mybir.AluOpType.is_gt)

    def bca(ap):
        return bass.AP(
            tensor=ap.tensor, offset=ap.offset, ap=list(ap.ap) + [[0, A]]
        )

    for sub in range(sb):
        xqv = xqs[sub][:].rearrange(
            "p ph (wo pw) ca -> p ph pw wo ca", pw=PW
        )
        cand = lambda p, _x=xqv: _x[:, p // PW, p % PW]
        nc.scalar.copy(oslab[:, sub], cand(0))
        dst = oslab[:, sub].rearrange("p w (c a) -> p w c a", a=A)
        for p in range(1, NP):
            nc.vector.copy_predicated(
                dst, bca(m[p - 1][:, sub]),
                cand(p).rearrange("p w (c a) -> p w c a", a=A),
            )
    oe = nc.scalar if cfg.get("out_act_ring") else nc.sync
    oe.dma_start(
        out=ov[r0 : r0 + 128, wq * sb * WCH : (wq + 1) * sb * WCH, :],
        in_=oslab,
    )


DEFAULT_CFG = dict(
    norm="tree_dve", contig=True, split4=True,
    load_bufs=5, sq_bufs=2, inplace=False, bufs=3,
    store_batch=2, wch=4,
)


def _build_bass(reps: int = 1, **overrides):
    """reps>1 repeats the whole per-core computation inside one NEFF —
    used by the timing harness to separate device time from launch/upload
    overhead ((T_reps - T_1) / (reps - 1))."""
    cfg = {**DEFAULT_CFG, **overrides}
    nc = bass.Bass()
    x = nc.dram_tensor("x", [BL, H, W, C, A], F32, kind="ExternalInput")
    out = nc.dram_tensor("out", [BL, Ho, Wo, C, A], F32, kind="ExternalOutput")

    # [(b ho)=256, ph=2, pw=2, wo=32, ca=256]; (b, ho) merges because the
    # b stride (H*W*C*A) equals 32 * the ho stride (PH*W*C*A).
    xv = x.rearrange(
        "b (ho ph) (wo pw) c a -> (b ho) ph pw wo (c a)", ph=PH, pw=PW
    )
    # contiguous-load view: [(b ho), ph, w, ca] with w the full-res column.
    cfg["xvc"] = x.rearrange(
        "b (ho ph) w c a -> (b ho) ph w (c a)", ph=PH
    )
    ov = out.rearrange("b ho wo c a -> (b ho) wo (c a)")  # [256, 32, 256]

    WCH = cfg["wch"]
    NWCH = Wo // WCH

    def _out_eng():
        if cfg.get("out_gps_ring"):
            return nc.gpsimd
        return nc.scalar if cfg.get("out_act_ring") else nc.sync

    with tile.TileContext(nc) as tc:
        with (
            tc.tile_pool(name="big", bufs=cfg["bufs"]) as big,
            tc.tile_pool(name="small", bufs=cfg["bufs"]) as small,
        ):
            sb = cfg.get("store_batch", 1)
            if cfg.get("loads_only") or cfg.get("dma_only"):
                sb = 1
            assert NWCH % sb == 0

            def _one_rep():
                for rt in range(NRT):
                    r0 = rt * 128
                    for wq in range(NWCH // sb):
                        if cfg.get("slab_v4") and sb > 1:
                            _slab_v4(nc, big, small, xv, ov, r0, wq, cfg)
                            continue
                        if cfg.get("slab_sel") and sb > 1:
                            _slab_sel(nc, big, small, xv, ov, r0, wq, cfg)
                            continue
                        if sb == 1:
                            _group(nc, big, small, xv, ov, r0, wq * WCH, cfg)
                            continue
                        # Batch sb groups' outputs into one slab; a single
                        # store then has sb*WCH*CA*4 B contiguous per row.
                        oslab = big.tile(
                            [128, sb, WCH, CA], F32, name="oslab",
                            bufs=cfg.get("slab_bufs", 2),
                        )
                        for sub in range(sb):
                            _group(
                                nc, big, small, xv, ov, r0,
                                (wq * sb + sub) * WCH, cfg,
                                dst_out=oslab[:, sub],
                            )
                        _out_eng().dma_start(
                            out=ov[
                                r0 : r0 + 128,
                                wq * sb * WCH : (wq + 1) * sb * WCH, :,
                            ],
                            in_=oslab,
                        )

            if cfg.get("hw_loop"):
                # reps = loop iterations; hw_unroll bodies per iteration.
                hw_u = cfg.get("hw_unroll", 1)
                if reps > 0:
                    with tc.For_i(0, reps):
                        for _ in range(hw_u):
                            _one_rep()
            else:
                for _rep in range(reps):
                    _one_rep()
    _split_multi_waits(nc)
    return nc


_NC_CACHE = None


def kernel(x: np.ndarray) -> np.ndarray:
    global _NC_CACHE
    assert x.shape == (B, H, W, C, A) and x.dtype == np.float32
    if _NC_CACHE is None:
        _NC_CACHE = _build_bass()
    nc = _NC_CACHE

    shards = [
        np.ascontiguousarray(x[i * BL : (i + 1) * BL]) for i in range(NCORES)
    ]
    in_maps = [{"x": s} for s in shards]
    res = run_bass_kernel_spmd(nc, in_maps, list(range(NCORES)))
    return np.concatenate([r["out"] for r in res.results], axis=0)

